# revision 3
# baseline (speedup 1.0000x reference)
"""SpecAugment (log-mel masking) Trainium2 kernel, v3.

Full inputs: x [64,128,3000] f32, f0/f_w/t0/t_w [64,2] i32.
out[b,f,t] = fill_b if (f in freq band) or (t in time band) else x[b,f,t],
fill_b = min over x[b].

Strategy: batch-shard B=64 across 8 cores (8 samples/core). rel-err
tolerance (2e-2) permits bf16 output, halving write traffic:
HBM/core = 12.3MB f32 in + 6.1MB bf16 out -> ~51us roofline.

Per sample (tiny mask params preprocessed on host into data vectors):
  - SWDGE cast-DMA x[b] f32 HBM -> bf16 SBUF (no compute pass for cast)
  - DVE tensor_tensor(min) halves + tensor_reduce -> colmin; tiny
    gather+reduce -> fill [1,1]; broadcast to [128,1] via tiny PE matmul
  - PE applies masks via PSUM accumulate, 512-col chunks:
      acc = I128 @ x_chunk + [ones; af]^T @ [tmB_chunk; ones]
    where tmB[t] = -1e30 if t time-masked, af[f] = -1e38 if f freq-masked
    (host data). Masked cells ~ -1e30/-1e38, unmasked exactly x.
  - ACT drains PSUM -> bf16 (the cast it would do anyway)
  - DVE 4x-mode tensor_scalar: out = acc max fill -- masked cells become
    exactly fill (valid since fill = min(x) <= x everywhere)
  - HWDGE DMA bf16 -> y[b]; host upcasts to f32
"""

import ml_dtypes
import numpy as np

import concourse.bacc as bacc
import concourse.bass as bass
import concourse.mybir as mybir
import concourse.tile as tile
import concourse.bass_utils as bass_utils

B, F, T = 64, 128, 3000
N_CORES = 8
BPC = B // N_CORES  # samples per core
F32 = mybir.dt.float32
BF16 = mybir.dt.bfloat16
H = T // 2
TH = T // 3  # third = 1000 cols = 2 PSUM banks

_cached = {}


def _build_nc():
    nc = bacc.Bacc("TRN2", target_bir_lowering=False, debug=False)
    x = nc.dram_tensor("x_sh", [BPC, F, T], F32, kind="ExternalInput")
    ident = nc.dram_tensor("ident", [F, F], BF16, kind="ExternalInput")
    # row0 = ones, row1 = -1e38 * freq_mask, per sample along columns
    mw = nc.dram_tensor("mw_sh", [2, BPC * F], BF16, kind="ExternalInput")
    # row0 = -1e30 * time_mask, row1 = ones, per sample along columns
    mr = nc.dram_tensor("mr_sh", [2, BPC * T], BF16, kind="ExternalInput")
    y = nc.dram_tensor("y_sh", [BPC, F, T], BF16, kind="ExternalOutput")

    xa, ya = x.ap(), y.ap()

    with tile.TileContext(nc) as tc:
        with (
            tc.tile_pool(name="xp", bufs=6) as xp,
            tc.tile_pool(name="op", bufs=4) as op,
            tc.tile_pool(name="thp", bufs=3) as thp,
            tc.tile_pool(name="small", bufs=6) as sp,
            tc.tile_pool(name="single", bufs=1) as single,
            tc.tile_pool(name="ps", bufs=3, space="PSUM") as psp,
            tc.tile_pool(name="ps_small", bufs=2, space="PSUM") as psps,
        ):
            ones_row = single.tile([1, F], F32)
            nc.vector.memset(ones_row, 1.0)
            one11 = single.tile([1, 1], F32)
            nc.vector.memset(one11, 1.0)
            it = single.tile([F, F], BF16)
            nc.sync.dma_start(out=it, in_=ident.ap())
            mw_all = single.tile([2, BPC * F], BF16)
            nc.sync.dma_start(out=mw_all, in_=mw.ap())
            mr_all = single.tile([2, BPC * T], BF16)
            nc.sync.dma_start(out=mr_all, in_=mr.ap())

            for b in range(BPC):
                # cast-DMA: f32 HBM -> bf16 SBUF (SWDGE)
                xc = xp.tile([F, T], BF16, tag="xc")
                nc.gpsimd.dma_start(out=xc, in_=xa[b])

                # per-sample min: DVE halve + reduce, cross-partition gather
                th = thp.tile([F, H], BF16, tag="th")
                nc.vector.tensor_tensor(
                    out=th, in0=xc[:, :H], in1=xc[:, H:], op=mybir.AluOpType.min
                )
                colmin = sp.tile([F, 1], F32, tag="colmin")
                nc.vector.tensor_reduce(
                    out=colmin, in_=th, axis=mybir.AxisListType.X,
                    op=mybir.AluOpType.min,
                )
                rowmin = sp.tile([1, F], F32, tag="rowmin")
                nc.gpsimd.dma_start(out=rowmin, in_=colmin)
                fill11 = sp.tile([1, 1], F32, tag="fill11")
                nc.vector.tensor_reduce(
                    out=fill11, in_=rowmin, axis=mybir.AxisListType.X,
                    op=mybir.AluOpType.min,
                )
                fill_row = sp.tile([1, F], F32, tag="fill_row")
                nc.scalar.mul(fill_row, ones_row, fill11)
                fill128_ps = psps.tile([F, 1], F32, tag="fill128_ps")
                nc.tensor.matmul(fill128_ps, fill_row, one11, start=True, stop=True)
                fill128 = sp.tile([F, 1], F32, tag="fill128")
                nc.scalar.copy(fill128, fill128_ps)

                # PE: acc = I@x + masks, per third (2 chunks of <=512)
                xf = op.tile([F, T], BF16, tag="xf")
                mwb = mw_all[:, b * F : (b + 1) * F]
                accs = []
                for j in range(3):
                    acc = psp.tile([F, TH], F32, tag="acc")
                    accs.append(acc)
                    for c0 in (0, 512):
                        cw = min(512, TH - c0)
                        col = j * TH + c0
                        nc.tensor.matmul(
                            acc[:, c0 : c0 + cw],
                            it,
                            xc[:, col : col + cw],
                            start=True,
                            stop=False,
                        )
                        nc.tensor.matmul(
                            acc[:, c0 : c0 + cw],
                            mwb,
                            mr_all[:, b * T + col : b * T + col + cw],
                            start=False,
                            stop=True,
                        )
                for j in range(3):
                    nc.scalar.copy(xf[:, j * TH : (j + 1) * TH], accs[j])

                # clamp: masked cells (~ -1e30/-1e38) -> fill
                nc.vector.tensor_scalar(
                    out=xf,
                    in0=xf,
                    scalar1=fill128,
                    scalar2=None,
                    op0=mybir.AluOpType.max,
                )
                nc.scalar.dma_start(out=ya[b], in_=xf)
    nc.compile()
    return nc


def _host_prep(f0, f_w, t0, t_w):
    fidx = np.arange(F, dtype=np.int32)
    tidx = np.arange(T, dtype=np.int32)
    fm = (
        (fidx[None, None, :] >= f0[:, :, None])
        & (fidx[None, None, :] < (f0 + f_w)[:, :, None])
    ).any(axis=1)  # [B,F] bool
    tm = (
        (tidx[None, None, :] >= t0[:, :, None])
        & (tidx[None, None, :] < (t0 + t_w)[:, :, None])
    ).any(axis=1)  # [B,T] bool
    tmb = (tm.astype(np.float32) * np.float32(-1e30)).astype(ml_dtypes.bfloat16)
    af = (fm.astype(np.float32) * np.float32(-1e38)).astype(ml_dtypes.bfloat16)
    return tmb, af


def _make_in_maps(x, f0, f_w, t0, t_w):
    x = np.ascontiguousarray(np.asarray(x, dtype=np.float32))
    tmb, af = _host_prep(
        np.asarray(f0), np.asarray(f_w), np.asarray(t0), np.asarray(t_w)
    )
    ident = np.eye(F, dtype=np.float32).astype(ml_dtypes.bfloat16)
    ones_f = np.ones(BPC * F, np.float32).astype(ml_dtypes.bfloat16)
    ones_t = np.ones(BPC * T, np.float32).astype(ml_dtypes.bfloat16)
    in_maps = []
    for c in range(N_CORES):
        s = slice(c * BPC, (c + 1) * BPC)
        mw = np.stack([ones_f, af[s].reshape(-1)])
        mr = np.stack([tmb[s].reshape(-1), ones_t])
        in_maps.append(
            {
                "x_sh": np.ascontiguousarray(x[s]),
                "ident": ident,
                "mw_sh": np.ascontiguousarray(mw),
                "mr_sh": np.ascontiguousarray(mr),
            }
        )
    return in_maps


def kernel(x, f0, f_w, t0, t_w, **_):
    in_maps = _make_in_maps(x, f0, f_w, t0, t_w)
    if "nc" not in _cached:
        _cached["nc"] = _build_nc()
    nc = _cached["nc"]
    res = bass_utils.run_bass_kernel_spmd(
        nc, in_maps, core_ids=list(range(N_CORES))
    )
    out = np.concatenate([np.asarray(r["y_sh"]) for r in res.results], axis=0)
    return out.astype(np.float32)


# revision 9
# speedup vs baseline: 1.1392x; 1.1392x over previous
"""SpecAugment (log-mel masking) Trainium2 kernel, v4.

Full inputs: x [64,128,3000] f32, f0/f_w/t0/t_w [64,2] i32.
out[b,f,t] = fill_b if (f in freq band) or (t in time band) else x[b,f,t],
fill_b = min over x[b].

Strategy: batch-shard B=64 across 8 cores (8 samples/core). rel-err
tolerance (2e-2) permits bf16 output, halving write traffic:
HBM/core = 12.3MB f32 in + 6.1MB bf16 out -> ~51us roofline.

Per sample (tiny mask params preprocessed on host into data vectors):
  - SWDGE cast-DMA x[b] f32 HBM -> bf16 SBUF (no compute pass for cast)
  - DVE tensor_tensor(min) halves + tensor_reduce -> colmin [128,1];
    tiny gather (sync queue) + reduce -> fill [1,1]
  - penalty[f,t] = nf[f] * (1e30*nt[t]) + fill * 1  (nf/nt = NOT-masked
    indicators, host data) via one K=2 PE matmul per 512-col chunk into
    PSUM; the fill row of the lhsT is written by the tiny ACT op that
    broadcasts fill11. ACT drains PSUM -> bf16.
  - DVE 2x-mode tensor_tensor: out = min(x, penalty) -- unmasked cells
    see min(x, 1e30) = x, masked see min(x, fill) = fill (fill = min(x))
  - HWDGE DMA bf16 -> y[b]; host upcasts to f32
The per-sample chain is software-pipelined 3 deep (load | reduce |
mask+store) so each engine's in-order stream never waits on a
same-iteration cross-engine result.
"""

import ml_dtypes
import numpy as np

import concourse.bacc as bacc
import concourse.bass as bass
import concourse.mybir as mybir
import concourse.tile as tile
import concourse.bass_utils as bass_utils

B, F, T = 64, 128, 3000
N_CORES = 8
BPC = B // N_CORES  # samples per core
F32 = mybir.dt.float32
BF16 = mybir.dt.bfloat16
H = T // 2
TH = T // 3  # third = 1000 cols = 2 PSUM banks

_cached = {}


def _build_nc():
    nc = bacc.Bacc("TRN2", target_bir_lowering=False, debug=False)
    x = nc.dram_tensor("x_sh", [BPC, F, T], F32, kind="ExternalInput")
    # row0 = zeros (overwritten with per-sample fill); row1 = 1 - freq_mask
    pw = nc.dram_tensor("pw_sh", [2, BPC * F], BF16, kind="ExternalInput")
    # row0 = ones; row1 = 1e30 * (1 - time_mask) per sample along columns
    nt = nc.dram_tensor("nt_sh", [2, BPC * T], BF16, kind="ExternalInput")
    y = nc.dram_tensor("y_sh", [BPC, F, T], BF16, kind="ExternalOutput")

    xa, ya = x.ap(), y.ap()

    with tile.TileContext(nc) as tc:
        with (
            tc.tile_pool(name="xp", bufs=5) as xp,
            tc.tile_pool(name="op", bufs=3) as op,
            tc.tile_pool(name="pp", bufs=3) as pp,
            tc.tile_pool(name="thp", bufs=3) as thp,
            tc.tile_pool(name="small", bufs=8) as sp,
            tc.tile_pool(name="single", bufs=1) as single,
            tc.tile_pool(name="ps", bufs=3, space="PSUM") as psp,
        ):
            ones_row = single.tile([1, F], F32)
            nc.vector.memset(ones_row, 1.0)
            pw_all = single.tile([2, BPC * F], BF16)
            nc.sync.dma_start(out=pw_all, in_=pw.ap())
            nt_all = single.tile([2, BPC * T], BF16)
            nc.sync.dma_start(out=nt_all, in_=nt.ap())

            xc = [None] * BPC
            th = [None] * BPC
            colmin = [None] * BPC
            rowmin = [None] * BPC
            fill11 = [None] * BPC

            # 3-stage software pipeline: i loads sample i, reduces sample
            # i-1, masks/stores sample i-2
            for i in range(BPC + 2):
                if i < BPC:
                    a = i
                    xc[a] = xp.tile([F, T], BF16, tag="xc", name=f"xc{a}")
                    # cast-DMA: f32 HBM -> bf16 SBUF (SWDGE)
                    nc.gpsimd.dma_start(out=xc[a], in_=xa[a])

                if 1 <= i <= BPC:
                    b = i - 1
                    th[b] = thp.tile([F, H], BF16, tag="th", name=f"th{b}")
                    nc.vector.tensor_tensor(
                        out=th[b], in0=xc[b][:, :H], in1=xc[b][:, H:],
                        op=mybir.AluOpType.min,
                    )
                    colmin[b] = sp.tile([F, 1], F32, tag="colmin", name=f"colmin{b}")
                    nc.vector.tensor_reduce(
                        out=colmin[b], in_=th[b], axis=mybir.AxisListType.X,
                        op=mybir.AluOpType.min,
                    )
                    rowmin[b] = sp.tile([1, F], F32, tag="rowmin", name=f"rowmin{b}")
                    nc.sync.dma_start(out=rowmin[b], in_=colmin[b])

                if 2 <= i:
                    c = i - 2
                    fill11[c] = sp.tile([1, 1], F32, tag="fill11", name=f"fill11{c}")
                    nc.vector.tensor_reduce(
                        out=fill11[c], in_=rowmin[c], axis=mybir.AxisListType.X,
                        op=mybir.AluOpType.min,
                    )
                    # write fill into row1 of this sample's penalty weights
                    nc.scalar.mul(
                        pw_all[0:1, c * F : (c + 1) * F], ones_row, fill11[c]
                    )
                    pwc = pw_all[:, c * F : (c + 1) * F]
                    pen = pp.tile([F, T], BF16, tag="pen")
                    for j in range(3):
                        acc = psp.tile([F, TH], F32, tag="acc")
                        for c0 in (0, 512):
                            cw = min(512, TH - c0)
                            off = c * T + j * TH + c0
                            nc.tensor.matmul(
                                acc[:, c0 : c0 + cw],
                                pwc,
                                nt_all[:, off : off + cw],
                                start=True,
                                stop=True,
                            )
                        nc.scalar.copy(pen[:, j * TH : (j + 1) * TH], acc)
                    xf = op.tile([F, T], BF16, tag="xf")
                    nc.vector.tensor_tensor(
                        out=xf, in0=xc[c], in1=pen, op=mybir.AluOpType.min
                    )
                    nc.scalar.dma_start(out=ya[c], in_=xf)
    nc.compile()
    return nc


def _host_prep(f0, f_w, t0, t_w):
    fidx = np.arange(F, dtype=np.int32)
    tidx = np.arange(T, dtype=np.int32)
    fm = (
        (fidx[None, None, :] >= f0[:, :, None])
        & (fidx[None, None, :] < (f0 + f_w)[:, :, None])
    ).any(axis=1)  # [B,F] bool
    tm = (
        (tidx[None, None, :] >= t0[:, :, None])
        & (tidx[None, None, :] < (t0 + t_w)[:, :, None])
    ).any(axis=1)  # [B,T] bool
    nf = (~fm).astype(np.float32).astype(ml_dtypes.bfloat16)  # [B,F]
    ntb = ((~tm).astype(np.float32) * np.float32(1e30)).astype(
        ml_dtypes.bfloat16
    )  # [B,T]
    return nf, ntb


def _make_in_maps(x, f0, f_w, t0, t_w):
    x = np.ascontiguousarray(np.asarray(x, dtype=np.float32))
    nf, ntb = _host_prep(
        np.asarray(f0), np.asarray(f_w), np.asarray(t0), np.asarray(t_w)
    )
    in_maps = []
    for c in range(N_CORES):
        s = slice(c * BPC, (c + 1) * BPC)
        pwm = np.zeros((2, BPC * F), np.float32).astype(ml_dtypes.bfloat16)
        pwm[1] = nf[s].reshape(-1)
        ntm = np.ones((2, BPC * T), np.float32).astype(ml_dtypes.bfloat16)
        ntm[1] = ntb[s].reshape(-1)
        in_maps.append(
            {
                "x_sh": np.ascontiguousarray(x[s]),
                "pw_sh": pwm,
                "nt_sh": ntm,
            }
        )
    return in_maps


def kernel(x, f0, f_w, t0, t_w, **_):
    in_maps = _make_in_maps(x, f0, f_w, t0, t_w)
    if "nc" not in _cached:
        _cached["nc"] = _build_nc()
    nc = _cached["nc"]
    res = bass_utils.run_bass_kernel_spmd(
        nc, in_maps, core_ids=list(range(N_CORES))
    )
    out = np.concatenate([np.asarray(r["y_sh"]) for r in res.results], axis=0)
    return out.astype(np.float32)
